# revision 1
# baseline (speedup 1.0000x reference)
"""RNN-T decoder + joint + loss as a Bass/Tile kernel on 8 TRN2 NeuronCores.

Strategy:
  - LSTM decoder (2x512, seq 51) replicated on all cores (it is
    weight-streaming bound; sharding over batch does not help it).
  - Joint network (B,T,U+1,512)xW + log-softmax-reduction sharded over T
    (25 frames per core) -- the FLOP-dominant part.
  - Per-(b,t,u) blank/label probabilities (shifted by e^C) all-gathered,
    then every core runs the RNN-T alpha recursion redundantly in
    probability space using tensor_tensor_scan (1st-order linear
    recurrence per frame), with periodic renormalization.
"""
import sys
sys.path.insert(0, '/opt/trn_rl_repo')

import numpy as np
import ml_dtypes

import concourse.bacc as bacc
import concourse.mybir as mybir
from concourse import bass_utils
from concourse.tile import TileContext

F32 = mybir.dt.float32
BF16 = mybir.dt.bfloat16
BF = ml_dtypes.bfloat16
AF = mybir.ActivationFunctionType
OP = mybir.AluOpType

B, T, U = 8, 200, 50
U1 = U + 1            # 51
D = 512
G = 2048              # 4 * D gates
NCORES = 8
TSH = T // NCORES     # 25 t per core
NTB = TSH * B         # 200 (t,b) pairs per core
ROWS = NTB * U1       # 10200 joint rows per core
RPAD = 10240          # padded to 80 m-tiles of 128
NMT = RPAD // 128     # 80
C_SHIFT = float(np.log(512.0))
EC = 512.0            # e^C_SHIFT
RENORM_EVERY = 25
# gate permutation: new chunk order (i, f, o, g_cell)
PERM = np.concatenate([np.arange(0, 512), np.arange(512, 1024),
                       np.arange(1536, 2048), np.arange(1024, 1536)])

_BUILD_CACHE = {}


def build(debug=False):
    nc = bacc.Bacc("TRN2", target_bir_lowering=False, debug=False,
                   num_devices=NCORES)

    # ---------------- I/O ----------------
    eysT_d = nc.dram_tensor("eysT", [D, U1 * B], BF16, kind="ExternalInput")
    wih0T_d = nc.dram_tensor("wih0T", [D, G], BF16, kind="ExternalInput")
    whh0T_d = nc.dram_tensor("whh0T", [D, G], BF16, kind="ExternalInput")
    wih1T_d = nc.dram_tensor("wih1T", [D, G], BF16, kind="ExternalInput")
    whh1T_d = nc.dram_tensor("whh1T", [D, G], BF16, kind="ExternalInput")
    bias0_d = nc.dram_tensor("bias0", [1, G], BF16, kind="ExternalInput")
    bias1_d = nc.dram_tensor("bias1", [1, G], BF16, kind="ExternalInput")
    hsT_d = nc.dram_tensor("hsT", [D, NTB], BF16, kind="ExternalInput")
    lencT_d = nc.dram_tensor("lencT", [D, D], BF16, kind="ExternalInput")
    ldecT_d = nc.dram_tensor("ldecT", [D, D], BF16, kind="ExternalInput")
    loutT_d = nc.dram_tensor("loutT", [D, D], BF16, kind="ExternalInput")
    lencb_d = nc.dram_tensor("lencb", [128, 4], F32, kind="ExternalInput")
    loutb_d = nc.dram_tensor("loutb", [1, D], BF16, kind="ExternalInput")
    ysidx_d = nc.dram_tensor("ysidx", [128, NMT], F32, kind="ExternalInput")
    ident8_d = nc.dram_tensor("ident8", [8, 8], BF16, kind="ExternalInput")

    loss_d = nc.dram_tensor("loss", [1, 1], F32, kind="ExternalOutput")
    if debug:
        h1T_dbg = nc.dram_tensor("h1T_dbg", [4, 128, U1 * B], BF16,
                                 kind="ExternalOutput")
        lp_dbg = nc.dram_tensor("lp_dbg", [2, RPAD], F32, kind="ExternalOutput")

    # internal DRAM
    xp0_d = nc.dram_tensor("xp0_i", [B, U1 * G], F32, kind="Internal")
    xp1_d = nc.dram_tensor("xp1_i", [B, U1 * G], F32, kind="Internal")
    lp_loc = nc.dram_tensor("lp_loc", [2 * RPAD], F32, kind="Internal")
    lp_full = nc.dram_tensor("lp_full", [NCORES * 2 * RPAD], F32,
                             kind="Internal", addr_space="Shared")

    with TileContext(nc) as tc:
        with tc.tile_pool(name="persist", bufs=1) as pp:
            # ---- persistent SBUF tensors ----
            hT1 = []
            for k in range(4):
                hT1.append(pp.tile([128, U1 * B], BF16, tag=f"hT1_{k}", name=f"hT1_{k}"))

            ldecT = []
            for k in range(4):
                t = pp.tile([128, D], BF16, tag=f"ldecT_{k}", name=f"ldecT_{k}")
                nc.sync.dma_start(t[:, :], ldecT_d.ap()[128 * k:128 * (k + 1), :])
                ldecT.append(t)
            loutT = []
            for k in range(4):
                t = pp.tile([128, D], BF16, tag=f"loutT_{k}", name=f"loutT_{k}")
                nc.sync.dma_start(t[:, :], loutT_d.ap()[128 * k:128 * (k + 1), :])
                loutT.append(t)
            lencb = pp.tile([128, 4], F32, tag="lencb", name="lencb")
            nc.sync.dma_start(lencb[:, :], lencb_d.ap())
            loutb = pp.tile([1, D], BF16, tag="loutb", name="loutb")
            nc.sync.dma_start(loutb[:, :], loutb_d.ap())
            ysidx = pp.tile([128, NMT], F32, tag="ysidx", name="ysidx")
            nc.sync.dma_start(ysidx[:, :], ysidx_d.ap())
            ident8 = pp.tile([8, 8], BF16, tag="ident8", name="ident8")
            nc.sync.dma_start(ident8[:, :], ident8_d.ap())
            onesb = pp.tile([1, 128], BF16, tag="onesb", name="onesb")
            nc.vector.memset(onesb[:, :], 1.0)
            iot = pp.tile([128, D], F32, tag="iot", name="iot")
            nc.gpsimd.iota(iot[:, :], pattern=[[1, D]], channel_multiplier=0,
                           allow_small_or_imprecise_dtypes=True)

            encT = [pp.tile([128, NTB], F32, tag=f"encT_{k}", name=f"encT_{k}") for k in range(4)]
            decT = [pp.tile([128, B * U1], F32, tag=f"decT_{k}", name=f"decT_{k}") for k in range(4)]

            # per-batch-element scalars for the final loss assembly
            mbuf = pp.tile([8, 8], F32, tag="mbuf", name="mbuf")
            nc.vector.memset(mbuf[:, :], 1.0)

            # ============ phase 0: xp0 batch + enc ============
            m_sizes = [128, 128, 128, 24]
            with tc.tile_pool(name="prep", bufs=1) as prp, \
                 tc.tile_pool(name="prep_ps", bufs=1, space="PSUM") as prps:
                hsT = []
                for k in range(4):
                    t = prp.tile([128, NTB], BF16, tag=f"hsT_{k}", name=f"hsT_{k}")
                    nc.sync.dma_start(t[:, :], hsT_d.ap()[128 * k:128 * (k + 1), :])
                    hsT.append(t)
                lencT = []
                for k in range(4):
                    t = prp.tile([128, D], BF16, tag=f"lencT_{k}", name=f"lencT_{k}")
                    nc.sync.dma_start(t[:, :], lencT_d.ap()[128 * k:128 * (k + 1), :])
                    lencT.append(t)
                eysT = []
                for k in range(4):
                    t = prp.tile([128, U1 * B], BF16, tag=f"eysT_{k}", name=f"eysT_{k}")
                    nc.sync.dma_start(t[:, :], eysT_d.ap()[128 * k:128 * (k + 1), :])
                    eysT.append(t)
                wih0 = []
                for k in range(4):
                    t = prp.tile([128, G], BF16, tag=f"wih0_{k}", name=f"wih0_{k}")
                    nc.sync.dma_start(t[:, :], wih0T_d.ap()[128 * k:128 * (k + 1), :])
                    wih0.append(t)
                bias0 = prp.tile([1, G], BF16, tag="bias0", name="bias0")
                nc.sync.dma_start(bias0[:, :], bias0_d.ap())

                for m in range(4):
                    mo, mr = 128 * m, m_sizes[m]
                    ps = prps.tile([128, G], F32, tag="xp_ps", name="xp_ps")
                    for n in range(4):
                        for k in range(4):
                            nc.tensor.matmul(
                                ps[0:mr, 512 * n:512 * (n + 1)],
                                lhsT=eysT[k][:, mo:mo + mr],
                                rhs=wih0[k][:, 512 * n:512 * (n + 1)],
                                start=(k == 0), stop=False)
                        nc.tensor.matmul(
                            ps[0:mr, 512 * n:512 * (n + 1)],
                            lhsT=onesb[0:1, 0:mr],
                            rhs=bias0[0:1, 512 * n:512 * (n + 1)],
                            start=False, stop=True)
                    # reshape rows (u*8+b) -> xp0_d[b, u*G+g] ; 8 DMAs
                    stg = prp.tile([128, G], F32, tag="xp_stg", name="xp_stg")
                    nc.vector.tensor_copy(stg[0:mr, :], ps[0:mr, :])
                    nu = mr // 8
                    u0 = mo // 8
                    for b in range(8):
                        nc.sync.dma_start(
                            xp0_d.ap()[b, G * u0:G * (u0 + nu)]
                                .rearrange("(u g) -> u g", u=nu),
                            stg[b:b + 8 * (nu - 1) + 1:8, :])

                # enc = hs @ lenc.T + lencb  (per j-chunk)
                for jc in range(4):
                    eps = prps.tile([128, NTB], F32, tag="enc_ps", name="enc_ps")
                    for k in range(4):
                        nc.tensor.matmul(eps[:, :], lhsT=lencT[k][:, 128 * jc:128 * (jc + 1)],
                                         rhs=hsT[k][:, :], start=(k == 0), stop=(k == 3))
                    nc.vector.tensor_scalar(out=encT[jc][:, :], in0=eps[:, :],
                                            scalar1=lencb[:, jc:jc + 1], scalar2=None,
                                            op0=OP.add)

            # ============ LSTM recurrence ============
            with tc.tile_pool(name="lstmw", bufs=1) as lwp, \
                 tc.tile_pool(name="step", bufs=2) as sp, \
                 tc.tile_pool(name="lstm_ps", bufs=1, space="PSUM") as lps, \
                 tc.tile_pool(name="tp_ps", bufs=2, space="PSUM") as tps, \
                 tc.tile_pool(name="xp1_ps", bufs=1, space="PSUM") as xps:

                whh = {}
                for l, wd in ((0, whh0T_d), (1, whh1T_d)):
                    whh[l] = []
                    for k in range(4):
                        t = lwp.tile([128, G], BF16, tag=f"whh{l}_{k}", name=f"whh{l}_{k}")
                        nc.sync.dma_start(t[:, :], wd.ap()[128 * k:128 * (k + 1), :])
                        whh[l].append(t)
                wih1 = []
                for k in range(4):
                    t = lwp.tile([128, G], BF16, tag=f"wih1_{k}", name=f"wih1_{k}")
                    nc.sync.dma_start(t[:, :], wih1T_d.ap()[128 * k:128 * (k + 1), :])
                    wih1.append(t)
                bias1 = lwp.tile([1, G], BF16, tag="bias1", name="bias1")
                nc.sync.dma_start(bias1[:, :], bias1_d.ap())
                hT0 = [lwp.tile([128, U1 * B], BF16, tag=f"hT0_{k}", name=f"hT0_{k}") for k in range(4)]
                hT = {0: hT0, 1: hT1}

                c_st = {l: lwp.tile([8, D], F32, tag=f"c{l}", name=f"c{l}") for l in (0, 1)}
                for l in (0, 1):
                    nc.vector.memset(c_st[l][:, :], 0.0)

                def lstm_step(l, u, xp_dram):
                    xp = sp.tile([8, G], F32, tag=f"xp{l}", name=f"xp{l}")
                    nc.sync.dma_start(
                        xp[:, :], xp_dram.ap()[:, G * u:G * (u + 1)])
                    if u > 0:
                        gp = lps.tile([128, 512], F32, tag=f"g{l}", name=f"g{l}")
                        for n in range(4):
                            for k in range(4):
                                nc.tensor.matmul(
                                    gp[32 * n:32 * n + 8, :],
                                    lhsT=hT[l][k][:, 8 * (u - 1):8 * u],
                                    rhs=whh[l][k][:, 512 * n:512 * (n + 1)],
                                    start=(k == 0), stop=(k == 3),
                                    tile_position=(0, 32 * n))
                        for n in range(4):
                            nc.vector.tensor_tensor(
                                out=xp[:, 512 * n:512 * (n + 1)],
                                in0=gp[32 * n:32 * n + 8, :],
                                in1=xp[:, 512 * n:512 * (n + 1)], op=OP.add)
                    a_ifo = sp.tile([8, 1536], F32, tag=f"aifo{l}", name=f"aifo{l}")
                    nc.scalar.activation(a_ifo[:, :], xp[:, 0:1536], AF.Sigmoid)
                    a_g = sp.tile([8, D], F32, tag=f"ag{l}", name=f"ag{l}")
                    nc.scalar.activation(a_g[:, :], xp[:, 1536:2048], AF.Tanh)
                    c2 = sp.tile([8, D], F32, tag=f"c2{l}", name=f"c2{l}")
                    nc.vector.tensor_tensor(out=c2[:, :], in0=a_ifo[:, 0:512],
                                            in1=a_g[:, :], op=OP.mult)
                    c = c_st[l]
                    if u == 0:
                        nc.vector.tensor_copy(c[:, :], c2[:, :])
                    else:
                        c1 = sp.tile([8, D], F32, tag=f"c1{l}", name=f"c1{l}")
                        nc.vector.tensor_tensor(out=c1[:, :], in0=a_ifo[:, 512:1024],
                                                in1=c[:, :], op=OP.mult)
                        nc.vector.tensor_tensor(out=c[:, :], in0=c1[:, :],
                                                in1=c2[:, :], op=OP.add)
                    thc = sp.tile([8, D], F32, tag=f"thc{l}", name=f"thc{l}")
                    nc.scalar.activation(thc[:, :], c[:, :], AF.Tanh)
                    hb = sp.tile([8, D], BF16, tag=f"hb{l}", name=f"hb{l}")
                    nc.vector.tensor_tensor(out=hb[:, :], in0=a_ifo[:, 1024:1536],
                                            in1=thc[:, :], op=OP.mult)
                    tp = tps.tile([128, 32], BF16, tag="tp", name="tp")
                    for k in range(4):
                        nc.tensor.transpose(tp[:, 8 * k:8 * (k + 1)],
                                            hb[:, 128 * k:128 * (k + 1)],
                                            ident8[:, :])
                        nc.vector.tensor_copy(hT[l][k][:, 8 * u:8 * (u + 1)],
                                              tp[:, 8 * k:8 * (k + 1)])

                ublocks = [(0, 16), (16, 32), (32, 48), (48, 51)]
                for (ub0, ub1) in ublocks:
                    for u in range(ub0, ub1):
                        lstm_step(0, u, xp0_d)
                    # xp1 batch for this block: xp1 = h0 @ wih1.T + bias1
                    mo, mr = 8 * ub0, 8 * (ub1 - ub0)
                    ps = xps.tile([128, G], F32, tag="xp1_ps", name="xp1_ps")
                    for n in range(4):
                        for k in range(4):
                            nc.tensor.matmul(
                                ps[0:mr, 512 * n:512 * (n + 1)],
                                lhsT=hT[0][k][:, mo:mo + mr],
                                rhs=wih1[k][:, 512 * n:512 * (n + 1)],
                                start=(k == 0), stop=False)
                        nc.tensor.matmul(
                            ps[0:mr, 512 * n:512 * (n + 1)],
                            lhsT=onesb[0:1, 0:mr],
                            rhs=bias1[0:1, 512 * n:512 * (n + 1)],
                            start=False, stop=True)
                    stg1 = sp.tile([128, G], F32, tag="xp1_stg", name="xp1_stg")
                    nc.vector.tensor_copy(stg1[0:mr, :], ps[0:mr, :])
                    nu = mr // 8
                    for b in range(8):
                        nc.sync.dma_start(
                            xp1_d.ap()[b, G * ub0:G * (ub0 + nu)]
                                .rearrange("(u g) -> u g", u=nu),
                            stg1[b:b + 8 * (nu - 1) + 1:8, :])
                    for u in range(ub0, ub1):
                        lstm_step(1, u, xp1_d)

            if debug:
                for k in range(4):
                    nc.sync.dma_start(h1T_dbg.ap()[k], hT1[k][:, :])

            # ============ dec projection ============
            with tc.tile_pool(name="dec_ps", bufs=2, space="PSUM") as dps:
                for jc in range(4):
                    ps = dps.tile([128, U1 * B], F32, tag="dec_ps", name="dec_ps")
                    for k in range(4):
                        nc.tensor.matmul(ps[:, :],
                                         lhsT=ldecT[k][:, 128 * jc:128 * (jc + 1)],
                                         rhs=hT1[k][:, :],
                                         start=(k == 0), stop=(k == 3))
                    # reorder columns (u*8+b) -> (b*51+u)
                    nc.vector.tensor_copy(
                        decT[jc][:, :].rearrange("p (b u) -> p b u", b=B),
                        ps[:, :].rearrange("p (u b) -> p b u", u=U1))

            # ============ joint ============
            with tc.tile_pool(name="joint", bufs=2) as jp, \
                 tc.tile_pool(name="jexp", bufs=3) as jep, \
                 tc.tile_pool(name="z1_pool", bufs=1) as z1p, \
                 tc.tile_pool(name="joint_ps", bufs=4, space="PSUM") as jps:

                z1T = [z1p.tile([128, RPAD], BF16, tag=f"z1T_{k}", name=f"z1T_{k}") for k in range(4)]
                # build z1 = tanh(enc + dec + lencb) ; 5 tl per block
                for jc in range(4):
                    for blk in range(5):
                        tl0, ntl = 5 * blk, 5
                        zs = jp.tile([128, ntl * B * U1], F32, tag="zs", name="zs")
                        in0 = encT[jc][:, 8 * tl0:8 * (tl0 + ntl)] \
                            .rearrange("p (t b) -> p t b", t=ntl) \
                            .unsqueeze(3).broadcast_to([128, ntl, B, U1])
                        in1 = decT[jc][:, :].rearrange("p (b u) -> p b u", b=B) \
                            .unsqueeze(1).broadcast_to([128, ntl, B, U1])
                        nc.vector.tensor_tensor(
                            out=zs[:, :].rearrange("p (t b u) -> p t b u", t=ntl, b=B),
                            in0=in0, in1=in1, op=OP.add)
                        nc.scalar.activation(
                            z1T[jc][:, 408 * tl0:408 * (tl0 + ntl)], zs[:, :],
                            AF.Tanh)
                # pad rows: zero them so exp() gives finite garbage we ignore
                for jc in range(4):
                    nc.vector.memset(z1T[jc][:, ROWS:RPAD], 0.0)

                rs_all = jp.tile([128, NMT], F32, tag="rs_all", name="rs_all")
                eb_all = jp.tile([128, NMT], F32, tag="eb_all", name="eb_all")
                el_all = jp.tile([128, NMT], F32, tag="el_all", name="el_all")

                for m in range(NMT):
                    mo = 128 * m
                    zp = jps.tile([128, 512], F32, tag="zp", name="zp")
                    for k in range(4):
                        nc.tensor.matmul(zp[:, :], lhsT=z1T[k][:, mo:mo + 128],
                                         rhs=loutT[k][:, :], start=(k == 0),
                                         stop=False)
                    nc.tensor.matmul(zp[:, :], lhsT=onesb[0:1, :],
                                     rhs=loutb[0:1, :], start=False, stop=True)
                    ez = jep.tile([128, 512], F32, tag="ez", name="ez")
                    nc.scalar.activation(ez[:, :], zp[:, :], AF.Exp,
                                         accum_out=rs_all[:, m:m + 1])
                    nc.vector.tensor_copy(eb_all[:, m:m + 1], ez[:, 0:1])
                    mask = jep.tile([128, 512], F32, tag="mask", name="mask")
                    nc.vector.tensor_tensor(
                        out=mask[:, :], in0=iot[:, :],
                        in1=ysidx[:, m:m + 1].broadcast_to([128, 512]),
                        op=OP.is_equal)
                    nc.vector.tensor_tensor(out=mask[:, :], in0=ez[:, :],
                                            in1=mask[:, :], op=OP.mult)
                    nc.vector.tensor_reduce(out=el_all[:, m:m + 1], in_=mask[:, :],
                                            axis=mybir.AxisListType.X, op=OP.add)

                # pb = eb * EC / rs ; pl = el * EC / rs
                rec = jp.tile([128, NMT], F32, tag="rec", name="rec")
                nc.vector.reciprocal(rec[:, :], rs_all[:, :])
                nc.vector.tensor_scalar(out=rec[:, :], in0=rec[:, :], scalar1=EC,
                                        scalar2=None, op0=OP.mult)
                nc.vector.tensor_tensor(out=eb_all[:, :], in0=eb_all[:, :],
                                        in1=rec[:, :], op=OP.mult)
                nc.vector.tensor_tensor(out=el_all[:, :], in0=el_all[:, :],
                                        in1=rec[:, :], op=OP.mult)
                nc.sync.dma_start(
                    lp_loc.ap()[0:RPAD].rearrange("(g p) -> p g", p=128),
                    eb_all[:, :])
                nc.sync.dma_start(
                    lp_loc.ap()[RPAD:2 * RPAD].rearrange("(g p) -> p g", p=128),
                    el_all[:, :])
                if debug:
                    nc.sync.dma_start(
                        lp_dbg.ap()[0].rearrange("(g p) -> p g", p=128), eb_all[:, :])
                    nc.sync.dma_start(
                        lp_dbg.ap()[1].rearrange("(g p) -> p g", p=128), el_all[:, :])

            # ============ all-gather ============
            nc.gpsimd.collective_compute(
                "AllGather", OP.bypass, replica_groups=[list(range(NCORES))],
                ins=[lp_loc.ap()], outs=[lp_full.ap()])

            # ============ DP ============
            with tc.tile_pool(name="dp", bufs=1) as dp:
                pb = dp.tile([8, T * U1], F32, tag="pb", name="pb")
                pl = dp.tile([8, T * U1], F32, tag="pl", name="pl")
                for r in range(NCORES):
                    base = r * 2 * RPAD
                    for (dst, off) in ((pb, 0), (pl, RPAD)):
                        nc.sync.dma_start(
                            dst[:, U1 * TSH * r:U1 * TSH * (r + 1)]
                                .rearrange("p (t u) -> p t u", t=TSH),
                            lp_full.ap()[base + off:base + off + ROWS]
                                .rearrange("(t b u) -> b t u", t=TSH, b=B))

                A = dp.tile([8, U1], F32, tag="A", name="A")
                Bt = dp.tile([8, U1], F32, tag="Bt", name="Bt")
                zz = dp.tile([8, U], F32, tag="zz", name="zz")
                nc.vector.memset(zz[:, :], 0.0)
                nc.vector.memset(A[:, 0:1], 1.0)
                nc.vector.tensor_tensor_scan(
                    out=A[:, 1:U1], data0=pl[:, 0:U], data1=zz[:, :],
                    initial=1.0, op0=OP.mult, op1=OP.add)
                nren = 0
                for t in range(1, T):
                    nc.vector.tensor_tensor(
                        out=Bt[:, :], in0=A[:, :],
                        in1=pb[:, U1 * (t - 1):U1 * t], op=OP.mult)
                    nc.vector.tensor_tensor_scan(
                        out=A[:, 1:U1], data0=pl[:, U1 * t:U1 * t + U],
                        data1=Bt[:, 1:U1], initial=Bt[:, 0:1],
                        op0=OP.mult, op1=OP.add)
                    nc.vector.tensor_copy(A[:, 0:1], Bt[:, 0:1])
                    if t % RENORM_EVERY == 0 and t < T - 1:
                        mx = dp.tile([8, 1], F32, tag="mx", name="mx")
                        nc.vector.tensor_reduce(out=mx[:, :], in_=A[:, :],
                                                axis=mybir.AxisListType.X,
                                                op=OP.max)
                        nc.vector.tensor_copy(mbuf[:, nren:nren + 1], mx[:, :])
                        rcp = dp.tile([8, 1], F32, tag="rcp", name="rcp")
                        nc.vector.reciprocal(rcp[:, :], mx[:, :])
                        nc.vector.tensor_scalar(out=A[:, :], in0=A[:, :],
                                                scalar1=rcp[:, 0:1], scalar2=None,
                                                op0=OP.mult)
                        nren += 1

                # q = A[:,U] * pb[:, (T-1)*U1 + U]
                nc.vector.tensor_tensor(
                    out=mbuf[:, 7:8], in0=A[:, U:U1],
                    in1=pb[:, U1 * (T - 1) + U:U1 * (T - 1) + U1], op=OP.mult)
                lg = dp.tile([8, 8], F32, tag="lg", name="lg")
                nc.scalar.activation(lg[:, :], mbuf[:, :], AF.Ln)
                ssum = dp.tile([8, 1], F32, tag="ssum", name="ssum")
                nc.vector.tensor_reduce(out=ssum[:, :], in_=lg[:, :],
                                        axis=mybir.AxisListType.X, op=OP.add)
                # transpose (8,1) -> (1,8) via 32x32 block transpose
                tt = dp.tile([32, 32], F32, tag="tt", name="tt")
                nc.vector.memset(tt[:, :], 0.0)
                nc.vector.tensor_copy(tt[0:8, 0:1], ssum[:, :])
                tu = dp.tile([32, 32], F32, tag="tu", name="tu")
                nc.vector.transpose(tu[:, :], tt[:, :])
                tot = dp.tile([1, 1], F32, tag="tot", name="tot")
                nc.vector.tensor_reduce(out=tot[:, :], in_=tu[0:1, :],
                                        axis=mybir.AxisListType.X, op=OP.add)
                # loss = 250*C - tot/8
                nc.vector.tensor_scalar(out=tot[:, :], in0=tot[:, :],
                                        scalar1=-0.125, scalar2=250.0 * C_SHIFT,
                                        op0=OP.mult, op1=OP.add)
                nc.sync.dma_start(loss_d.ap(), tot[:, :])

    nc.compile()
    return nc


def _to_bf(x):
    return np.ascontiguousarray(x).astype(BF)


def _host_prep(inputs):
    hs_pad = np.asarray(inputs["hs_pad"], np.float32)
    ys_pad = np.asarray(inputs["ys_pad"])
    embed_w = np.asarray(inputs["embed_w"], np.float32)

    Emb = embed_w.copy()
    Emb[0] = 0.0
    ys_in = np.concatenate([np.zeros((B, 1), ys_pad.dtype), ys_pad], axis=1)
    eys = Emb[ys_in]                                   # (B, U1, D)
    eysT = np.ascontiguousarray(eys.transpose(2, 1, 0)).reshape(D, U1 * B)

    def wT(w):
        return np.ascontiguousarray(np.asarray(w, np.float32)[PERM].T)

    common = {
        "eysT": _to_bf(eysT),
        "wih0T": _to_bf(wT(inputs["w_ih0"])),
        "whh0T": _to_bf(wT(inputs["w_hh0"])),
        "wih1T": _to_bf(wT(inputs["w_ih1"])),
        "whh1T": _to_bf(wT(inputs["w_hh1"])),
        "bias0": _to_bf((np.asarray(inputs["b_ih0"], np.float32)
                         + np.asarray(inputs["b_hh0"], np.float32))[PERM][None, :]),
        "bias1": _to_bf((np.asarray(inputs["b_ih1"], np.float32)
                         + np.asarray(inputs["b_hh1"], np.float32))[PERM][None, :]),
        "lencT": _to_bf(np.asarray(inputs["lin_enc_w"], np.float32).T),
        "ldecT": _to_bf(np.asarray(inputs["lin_dec_w"], np.float32).T),
        "loutT": _to_bf(np.asarray(inputs["lin_out_w"], np.float32).T),
        "lencb": np.ascontiguousarray(
            np.asarray(inputs["lin_enc_b"], np.float32).reshape(4, 128).T),
        "loutb": _to_bf(np.asarray(inputs["lin_out_b"], np.float32)[None, :]),
        "ident8": _to_bf(np.eye(8, dtype=np.float32)),
    }

    # label index per padded row r = (tl*8+b)*51 + u  (same for every core)
    r = np.arange(RPAD)
    bb = (r // U1) % 8
    uu = r % U1
    vals = np.where((uu < U) & (r < ROWS), ys_pad[bb, np.minimum(uu, U - 1)], -1.0)
    common["ysidx"] = np.ascontiguousarray(
        vals.reshape(NMT, 128).T.astype(np.float32))

    in_maps = []
    for c in range(NCORES):
        hs = hs_pad[:, TSH * c:TSH * (c + 1), :]       # (B, 25, D)
        hsT = np.ascontiguousarray(hs.transpose(2, 1, 0)).reshape(D, NTB)
        m = dict(common)
        m["hsT"] = _to_bf(hsT)
        in_maps.append(m)
    return in_maps


def kernel(**inputs):
    if "nc" not in _BUILD_CACHE:
        _BUILD_CACHE["nc"] = build(debug=False)
    nc = _BUILD_CACHE["nc"]
    in_maps = _host_prep(inputs)
    res = bass_utils.run_bass_kernel_spmd(nc, in_maps,
                                          core_ids=list(range(NCORES)))
    return np.float32(res.results[0]["loss"][0, 0])


if __name__ == "__main__":
    dat = np.load("/root/problem/inputs.npz")
    out = kernel(**{k: dat[k] for k in dat.files})
    exp = float(np.load("/root/problem/expected.npy"))
    rel = abs(float(out) - exp) / abs(exp)
    print("loss =", out, "expected =", exp, "rel err =", rel)



# revision 15
# speedup vs baseline: 2.2198x; 2.2198x over previous
"""RNN-T decoder + joint + loss as a Bass/Tile kernel on 8 TRN2 NeuronCores.

v2 design:
  - LSTM decoder in FLIPPED layout: gates live as [128 gate-partitions, 8
    batch] via lhsT=whh-chunk matmuls (fp8 weights+h, x64 scaling, FWL),
    so activations/c-updates run on 128 lanes.  xp projections stay in
    SBUF (no DRAM roundtrip).
  - Joint sharded over T (25 frames/core), u-major row order
    r = u*200 + t*8 + b.  z1=tanh(enc+dec) built per-u (gpsimd add +
    scalar tanh -> fp8 ring), matmul vs lin_out in fp8 DoubleRow,
    exp on scalar, label-prob extraction in ONE fused vector
    scalar_tensor_tensor (iota==ys)*ez with accumulate; denominator
    (softmax with lin_out bias) via gpsimd stt vs exp(bias).
  - All-gather of blank/label prob tables (bf16), then a u-sweep DP:
    per u-column one fused rescale-multiply + one 200-long
    tensor_tensor_scan over t (prob space, per-column renorm).
"""
import sys
sys.path.insert(0, '/opt/trn_rl_repo')

import numpy as np
import ml_dtypes

import concourse.bacc as bacc
import concourse.mybir as mybir
from concourse import bass_utils
from concourse.tile import TileContext

F32 = mybir.dt.float32
BF16 = mybir.dt.bfloat16
FP8 = mybir.dt.float8e4
BF = ml_dtypes.bfloat16
F8 = ml_dtypes.float8_e4m3
AF = mybir.ActivationFunctionType
OP = mybir.AluOpType
DR = mybir.MatmulPerfMode.DoubleRow

B, T, U = 8, 200, 50
U1 = U + 1            # 51
D = 512
NCORES = 8
TSH = T // NCORES     # 25 t per core
NTB = TSH * B         # 200 rows per u per core
ROWS = NTB * U1       # 10200 joint rows per core
RPAD = 10240
NMT = RPAD // 128     # 80 m-tiles
RING = 2048           # z1 ring columns (16 slots of 128)
EC = 512.0
C_SHIFT = float(np.log(512.0))
WS = 64.0             # fp8 weight pre-scale
# gate permutation: chunk order (i, f, o, g_cell)
PERM = np.concatenate([np.arange(0, 512), np.arange(512, 1024),
                       np.arange(1536, 2048), np.arange(1024, 1536)])
UBLK = [(0, 13), (13, 26), (26, 39), (39, 51)]

_BUILD_CACHE = {}


def build(debug=False):
    nc = bacc.Bacc("TRN2", target_bir_lowering=False, debug=False,
                   num_devices=NCORES)

    # ---------------- I/O ----------------
    eysT_d = nc.dram_tensor("eysT", [128, 4 * U1 * B], FP8, kind="ExternalInput")
    wih0_d = nc.dram_tensor("wih0", [128, 8192], FP8, kind="ExternalInput")
    whh0_d = nc.dram_tensor("whh0", [128, 8192], FP8, kind="ExternalInput")
    wih1_d = nc.dram_tensor("wih1", [128, 8192], FP8, kind="ExternalInput")
    whh1_d = nc.dram_tensor("whh1", [128, 8192], FP8, kind="ExternalInput")
    bias0_d = nc.dram_tensor("bias0", [128, 16], F32, kind="ExternalInput")
    bias1_d = nc.dram_tensor("bias1", [128, 16], F32, kind="ExternalInput")
    hsT_d = nc.dram_tensor("hsT", [128, 4 * NTB], BF16, kind="ExternalInput")
    lencT_d = nc.dram_tensor("lencT", [128, 2048], BF16, kind="ExternalInput")
    ldecT_d = nc.dram_tensor("ldecT", [128, 2048], FP8, kind="ExternalInput")
    loutT_d = nc.dram_tensor("loutT", [128, 2048], FP8, kind="ExternalInput")
    lencb_d = nc.dram_tensor("lencb", [128, 4], F32, kind="ExternalInput")
    ysidx_d = nc.dram_tensor("ysidx", [128, NMT], F32, kind="ExternalInput")
    eblab_d = nc.dram_tensor("eblab", [128, NMT], F32, kind="ExternalInput")
    ebc_d = nc.dram_tensor("ebc", [128, 1], F32, kind="ExternalInput")
    ebrep_d = nc.dram_tensor("ebrep", [128, D], F32, kind="ExternalInput")

    loss_d = nc.dram_tensor("loss", [1, 1], F32, kind="ExternalOutput")
    if debug:
        h1_dbg = nc.dram_tensor("h1_dbg", [128, 4 * U1 * B], FP8,
                                kind="ExternalOutput")
        dec_dbg = nc.dram_tensor("dec_dbg", [128, 4 * U1 * B], F32,
                                 kind="ExternalOutput")
        pb_dbg = nc.dram_tensor("pb_dbg", [8, 1 + ROWS], BF16,
                                kind="ExternalOutput")
        pl_dbg = nc.dram_tensor("pl_dbg", [8, 1 + ROWS], BF16,
                                kind="ExternalOutput")

    lp_loc = nc.dram_tensor("lp_loc", [2 * RPAD], BF16, kind="Internal")
    lp_full = nc.dram_tensor("lp_full", [NCORES * 2 * RPAD], BF16,
                             kind="Internal", addr_space="Shared")

    with TileContext(nc) as tc:
        with tc.tile_pool(name="persist", bufs=1) as pp:
            # ---- persistent SBUF ----
            whh = {}
            wih = {}
            biasT = {}
            for l, (wd, hd, bd) in {0: (wih0_d, whh0_d, bias0_d),
                                    1: (wih1_d, whh1_d, bias1_d)}.items():
                wih[l] = pp.tile([128, 8192], FP8, tag=f"wih{l}", name=f"wih{l}")
                whh[l] = pp.tile([128, 8192], FP8, tag=f"whh{l}", name=f"whh{l}")
                biasT[l] = pp.tile([128, 16], F32, tag=f"biasT{l}", name=f"biasT{l}")
            eysT = pp.tile([128, 4 * U1 * B], FP8, tag="eysT", name="eysT")
            nc.sync.dma_start(eysT[:, :], eysT_d.ap())
            nc.sync.dma_start(wih[0][:, :], wih0_d.ap())
            nc.sync.dma_start(biasT[0][:, :], bias0_d.ap())
            nc.sync.dma_start(whh[0][:, :], whh0_d.ap())
            nc.sync.dma_start(wih[1][:, :], wih1_d.ap())
            nc.sync.dma_start(biasT[1][:, :], bias1_d.ap())
            nc.sync.dma_start(whh[1][:, :], whh1_d.ap())

            xpT = {l: pp.tile([128, 16 * U1 * B], BF16, tag=f"xpT{l}",
                              name=f"xpT{l}") for l in (0, 1)}
            hist = {l: pp.tile([128, 4 * U1 * B], FP8, tag=f"hist{l}",
                               name=f"hist{l}") for l in (0, 1)}
            cst = {l: pp.tile([128, 32], F32, tag=f"cst{l}", name=f"cst{l}")
                   for l in (0, 1)}

            ldecT = pp.tile([128, 2048], FP8, tag="ldecT", name="ldecT")
            nc.sync.dma_start(ldecT[:, :], ldecT_d.ap())
            loutT = pp.tile([128, 2048], FP8, tag="loutT", name="loutT")
            nc.sync.dma_start(loutT[:, :], loutT_d.ap())

            encT = pp.tile([128, 4 * NTB], F32, tag="encT", name="encT")
            decT = pp.tile([128, 4 * U1 * B], F32, tag="decT", name="decT")
            z1R = pp.tile([128, 4 * RING], FP8, tag="z1R", name="z1R")

            ysidx = pp.tile([128, NMT], F32, tag="ysidx", name="ysidx")
            nc.sync.dma_start(ysidx[:, :], ysidx_d.ap())
            eblab = pp.tile([128, NMT], F32, tag="eblab", name="eblab")
            nc.sync.dma_start(eblab[:, :], eblab_d.ap())
            ebc = pp.tile([128, 1], F32, tag="ebc", name="ebc")
            nc.sync.dma_start(ebc[:, :], ebc_d.ap())
            ebrep = pp.tile([128, D], F32, tag="ebrep", name="ebrep")
            nc.sync.dma_start(ebrep[:, :], ebrep_d.ap())
            iot = pp.tile([128, D], F32, tag="iot", name="iot")
            nc.gpsimd.iota(iot[:, :], pattern=[[1, D]], channel_multiplier=0,
                           allow_small_or_imprecise_dtypes=True)

            rs_all = pp.tile([128, NMT], F32, tag="rs_all", name="rs_all")
            eb_all = pp.tile([128, NMT], F32, tag="eb_all", name="eb_all")
            el_all = pp.tile([128, NMT], F32, tag="el_all", name="el_all")

            # convenient multi-dim views
            whh3 = {l: whh[l][:, :].rearrange("p (k g) -> p k g", k=4)
                    for l in (0, 1)}
            wih3 = {l: wih[l][:, :].rearrange("p (k g) -> p k g", k=4)
                    for l in (0, 1)}
            eysT3 = eysT[:, :].rearrange("p (k x) -> p k x", k=4)
            xpT4 = {l: xpT[l][:, :].rearrange("p (g u b) -> p g u b",
                                              g=16, u=U1) for l in (0, 1)}
            hist4 = {l: hist[l][:, :].rearrange("p (d u b) -> p d u b",
                                                d=4, u=U1) for l in (0, 1)}
            encT3 = encT[:, :].rearrange("p (j x) -> p j x", j=4)
            decT4 = decT[:, :].rearrange("p (j u b) -> p j u b", j=4, u=U1)
            z1R3 = z1R[:, :].rearrange("p (j c) -> p j c", j=4)
            ldecT3 = ldecT[:, :].rearrange("p (k j) -> p k j", k=4)
            loutT3 = loutT[:, :].rearrange("p (k v) -> p k v", k=4)

            # ============ phase 0: xp0 + enc ============
            with tc.tile_pool(name="prep", bufs=1) as prp, \
                 tc.tile_pool(name="prep_ps", bufs=2, space="PSUM") as prps:
                # xp0 = eys @ wih0.T (flipped): per gate-chunk
                for gc in range(16):
                    ps = prps.tile([128, U1 * B], F32, tag="xp0_ps", name="xp0_ps")
                    for k in range(4):
                        nc.tensor.matmul(
                            ps[:, :], lhsT=wih3[0][:, k, 128 * gc:128 * (gc + 1)],
                            rhs=eysT3[:, k, :], start=(k == 0), stop=(k == 3))
                    nc.scalar.activation(
                        xpT[0][:, U1 * B * gc:U1 * B * (gc + 1)], ps[:, :],
                        AF.Identity, bias=biasT[0][:, gc:gc + 1], scale=1.0 / WS)

                hsT = prp.tile([128, 4 * NTB], BF16, tag="hsT", name="hsT")
                nc.sync.dma_start(hsT[:, :], hsT_d.ap())
                lencT = prp.tile([128, 2048], BF16, tag="lencT", name="lencT")
                nc.sync.dma_start(lencT[:, :], lencT_d.ap())
                lencb = prp.tile([128, 4], F32, tag="lencb", name="lencb")
                nc.sync.dma_start(lencb[:, :], lencb_d.ap())
                hsT3 = hsT[:, :].rearrange("p (k x) -> p k x", k=4)
                lencT3 = lencT[:, :].rearrange("p (k j) -> p k j", k=4)
                for jc in range(4):
                    ps = prps.tile([128, NTB], F32, tag="enc_ps", name="enc_ps")
                    for k in range(4):
                        nc.tensor.matmul(
                            ps[:, :], lhsT=lencT3[:, k, 128 * jc:128 * (jc + 1)],
                            rhs=hsT3[:, k, :], start=(k == 0), stop=(k == 3))
                    nc.scalar.activation(
                        encT[:, NTB * jc:NTB * (jc + 1)], ps[:, :],
                        AF.Identity, bias=lencb[:, jc:jc + 1])

            # ============ LSTM + joint (pipelined waves) ============
            with tc.tile_pool(name="sp", bufs=3) as sp, \
                 tc.tile_pool(name="zsp", bufs=2) as zsp, \
                 tc.tile_pool(name="ezp", bufs=3) as ezp, \
                 tc.tile_pool(name="gp_ps", bufs=2, space="PSUM") as gps, \
                 tc.tile_pool(name="blk_ps", bufs=2, space="PSUM") as bps, \
                 tc.tile_pool(name="zp_ps", bufs=2, space="PSUM") as zps:

                def lstm_step(l, u):
                    if u > 0:
                        gp = gps.tile([128, 128], F32, tag=f"gp{l}", name=f"gp{l}")
                        for gc in range(16):
                            for k in range(4):
                                nc.tensor.matmul(
                                    gp[:, 8 * gc:8 * (gc + 1)],
                                    lhsT=whh3[l][:, k, 128 * gc:128 * (gc + 1)],
                                    rhs=hist4[l][:, k, u - 1, :],
                                    start=(k == 0), stop=(k == 3))
                        gs = sp.tile([128, 128], F32, tag=f"gs{l}", name=f"gs{l}")
                        nc.vector.scalar_tensor_tensor(
                            out=gs[:, :].rearrange("p (g b) -> p g b", g=16),
                            in0=gp[:, :].rearrange("p (g b) -> p g b", g=16),
                            scalar=1.0 / WS,
                            in1=xpT4[l][:, :, u, :],
                            op0=OP.mult, op1=OP.add)
                        gsap = gs[:, :]
                    else:
                        gsap = None
                    sg = sp.tile([128, 128], F32, tag=f"sg{l}", name=f"sg{l}")
                    if u > 0:
                        nc.scalar.activation(sg[:, 0:96], gsap[:, 0:96], AF.Sigmoid)
                        nc.scalar.activation(sg[:, 96:128], gsap[:, 96:128], AF.Tanh)
                    else:
                        nc.scalar.activation(
                            sg[:, 0:96].rearrange("p (g b) -> p g b", g=12),
                            xpT4[l][:, 0:12, 0, :], AF.Sigmoid)
                        nc.scalar.activation(
                            sg[:, 96:128].rearrange("p (g b) -> p g b", g=4),
                            xpT4[l][:, 12:16, 0, :], AF.Tanh)
                    c = cst[l]
                    if u == 0:
                        nc.vector.tensor_tensor(out=c[:, :], in0=sg[:, 0:32],
                                                in1=sg[:, 96:128], op=OP.mult)
                    else:
                        t1 = sp.tile([128, 32], F32, tag=f"t1{l}", name=f"t1{l}")
                        nc.vector.tensor_tensor(out=t1[:, :], in0=sg[:, 32:64],
                                                in1=c[:, :], op=OP.mult)
                        t2 = sp.tile([128, 32], F32, tag=f"t2{l}", name=f"t2{l}")
                        nc.vector.tensor_tensor(out=t2[:, :], in0=sg[:, 0:32],
                                                in1=sg[:, 96:128], op=OP.mult)
                        nc.vector.tensor_tensor(out=c[:, :], in0=t1[:, :],
                                                in1=t2[:, :], op=OP.add)
                    thc = sp.tile([128, 32], F32, tag=f"thc{l}", name=f"thc{l}")
                    nc.scalar.activation(thc[:, :], c[:, :], AF.Tanh)
                    # h = sig(o)*tanh(c) -> straight into fp8 history
                    nc.vector.tensor_tensor(
                        out=hist4[l][:, :, u, :],
                        in0=sg[:, 64:96].rearrange("p (d b) -> p d b", d=4),
                        in1=thc[:, :].rearrange("p (d b) -> p d b", d=4),
                        op=OP.mult)

                def xp1_block(u0, u1):
                    nn = 8 * (u1 - u0)
                    for gc in range(16):
                        ps = bps.tile([128, 8 * 13], F32, tag="blk_ps", name="blk_ps")
                        for k in range(4):
                            nc.tensor.matmul(
                                ps[:, 0:nn],
                                lhsT=wih3[1][:, k, 128 * gc:128 * (gc + 1)],
                                rhs=hist4[0][:, k, u0:u1, :],
                                start=(k == 0), stop=(k == 3))
                        nc.scalar.activation(
                            xpT4[1][:, gc, u0:u1, :],
                            ps[:, 0:nn].rearrange("p (u b) -> p u b", b=8),
                            AF.Identity, bias=biasT[1][:, gc:gc + 1],
                            scale=1.0 / WS)

                def dec_block(u0, u1):
                    nn = 8 * (u1 - u0)
                    for jc in range(4):
                        ps = bps.tile([128, 8 * 13], F32, tag="blk_ps", name="blk_ps")
                        for k in range(4):
                            nc.tensor.matmul(
                                ps[:, 0:nn],
                                lhsT=ldecT3[:, k, 128 * jc:128 * (jc + 1)],
                                rhs=hist4[1][:, k, u0:u1, :],
                                start=(k == 0), stop=(k == 3))
                        nc.scalar.activation(
                            decT4[:, jc, u0:u1, :],
                            ps[:, 0:nn].rearrange("p (u b) -> p u b", b=8),
                            AF.Identity, scale=1.0 / WS)

                tile_state = {"next_m": 0}

                def joint_tile(m):
                    slot = m % 16
                    c0 = 128 * slot
                    zp = zps.tile([128, D], F32, tag="zp", name="zp")
                    for jp in (0, 2):
                        nc.tensor.matmul(
                            zp[:, :], lhsT=z1R3[:, jp:jp + 2, c0:c0 + 128],
                            rhs=loutT3[:, jp:jp + 2, :],
                            start=(jp == 0), stop=(jp == 2), perf_mode=DR)
                    ez = ezp.tile([128, D], F32, tag="ez", name="ez")
                    nc.scalar.activation(ez[:, :], zp[:, :], AF.Exp,
                                         scale=1.0 / WS)
                    scr = ezp.tile([128, D], F32, tag="scr", name="scr")
                    nc.vector.scalar_tensor_tensor(
                        out=scr[:, :], in0=ez[:, :], scalar=1.0, in1=ebrep[:, :],
                        op0=OP.mult, op1=OP.mult,
                        accum_out=rs_all[:, m:m + 1])
                    scr2 = ezp.tile([128, D], F32, tag="scr2", name="scr2")
                    nc.vector.scalar_tensor_tensor(
                        out=scr2[:, :], in0=iot[:, :], scalar=ysidx[:, m:m + 1],
                        in1=ez[:, :], op0=OP.is_equal, op1=OP.mult,
                        accum_out=el_all[:, m:m + 1])
                    nc.vector.tensor_copy(eb_all[:, m:m + 1], ez[:, 0:1])

                def joint_u(u):
                    # rows within a u-block are (b, t): r = u*200 + b*25 + t
                    zs = zsp.tile([128, 4 * NTB], F32, tag="zs", name="zs")
                    nc.vector.tensor_tensor(
                        out=zs[:, :].rearrange("p (j b t) -> p j b t",
                                               j=4, b=8),
                        in0=encT3[:, :, :].rearrange("p j (b t) -> p j b t",
                                                     b=8),
                        in1=decT4[:, :, u, :].unsqueeze(3).broadcast_to(
                            [128, 4, 8, TSH]),
                        op=OP.add)
                    zs3 = zs[:, :].rearrange("p (j x) -> p j x", j=4)
                    c0 = (u * NTB) % RING
                    seg1 = min(RING - c0, NTB)
                    nc.scalar.activation(z1R3[:, :, c0:c0 + seg1],
                                         zs3[:, :, 0:seg1], AF.Tanh)
                    if seg1 < NTB:
                        nc.scalar.activation(z1R3[:, :, 0:NTB - seg1],
                                             zs3[:, :, seg1:NTB], AF.Tanh)
                    while (128 * tile_state["next_m"] + 127) // NTB <= u:
                        joint_tile(tile_state["next_m"])
                        tile_state["next_m"] += 1

                # ---- wave schedule ----
                for u in range(*UBLK[0]):
                    lstm_step(0, u)
                xp1_block(*UBLK[0])
                for i in range(13):
                    lstm_step(0, UBLK[1][0] + i)
                    lstm_step(1, UBLK[0][0] + i)
                xp1_block(*UBLK[1])
                for w in (2, 3):
                    dec_block(*UBLK[w - 2])
                    a0, a1 = UBLK[w]
                    p0, p1 = UBLK[w - 1]
                    j0, j1 = UBLK[w - 2]
                    for i in range(13):
                        if a0 + i < a1:
                            lstm_step(0, a0 + i)
                        if p0 + i < p1:
                            lstm_step(1, p0 + i)
                        if j0 + i < j1:
                            joint_u(j0 + i)
                    xp1_block(a0, a1)
                dec_block(*UBLK[2])
                for i in range(13):
                    if UBLK[3][0] + i < UBLK[3][1]:
                        lstm_step(1, UBLK[3][0] + i)
                    if UBLK[2][0] + i < UBLK[2][1]:
                        joint_u(UBLK[2][0] + i)
                dec_block(*UBLK[3])
                for u in range(*UBLK[3]):
                    joint_u(u)
                # pad rows 10200..10240 -> zero z1, then last tile
                pc0 = ROWS % RING
                nc.vector.memset(z1R3[:, :, pc0:pc0 + (RPAD - ROWS)], 0.0)
                while tile_state["next_m"] < NMT:
                    joint_tile(tile_state["next_m"])
                    tile_state["next_m"] += 1

            if debug:
                nc.sync.dma_start(h1_dbg.ap(), hist[1][:, :])
                nc.sync.dma_start(dec_dbg.ap(), decT[:, :])

            # ============ pb/pl tables + all-gather ============
            with tc.tile_pool(name="fin", bufs=1) as fp:
                rcp = fp.tile([128, NMT], F32, tag="rcp", name="rcp")
                nc.vector.reciprocal(rcp[:, :], rs_all[:, :])
                pbf = fp.tile([128, NMT], BF16, tag="pbf", name="pbf")
                nc.vector.scalar_tensor_tensor(
                    out=pbf[:, :], in0=eb_all[:, :], scalar=ebc[:, 0:1],
                    in1=rcp[:, :], op0=OP.mult, op1=OP.mult)
                el2 = fp.tile([128, NMT], F32, tag="el2", name="el2")
                nc.vector.tensor_tensor(out=el2[:, :], in0=el_all[:, :],
                                        in1=eblab[:, :], op=OP.mult)
                plf = fp.tile([128, NMT], BF16, tag="plf", name="plf")
                nc.vector.tensor_tensor(out=plf[:, :], in0=el2[:, :],
                                        in1=rcp[:, :], op=OP.mult)
                nc.sync.dma_start(
                    lp_loc.ap()[0:RPAD].rearrange("(g p) -> p g", p=128),
                    pbf[:, :])
                nc.sync.dma_start(
                    lp_loc.ap()[RPAD:2 * RPAD].rearrange("(g p) -> p g", p=128),
                    plf[:, :])

                nc.gpsimd.collective_compute(
                    "AllGather", OP.bypass,
                    replica_groups=[list(range(NCORES))],
                    ins=[lp_loc.ap()], outs=[lp_full.ap()])

                pbt = fp.tile([8, 1 + T * U1], BF16, tag="pbt", name="pbt")
                plt = fp.tile([8, 1 + T * U1], BF16, tag="plt", name="plt")
                nc.vector.memset(pbt[:, 0:1], 0.0)
                nc.vector.memset(plt[:, 0:1], 0.0)
                for r in range(NCORES):
                    base = r * 2 * RPAD
                    for (dst, off) in ((pbt, 0), (plt, RPAD)):
                        nc.sync.dma_start(
                            dst[:, 1:].rearrange("p (u t) -> p u t", u=U1)
                                [:, :, TSH * r:TSH * (r + 1)],
                            lp_full.ap()[base + off:base + off + ROWS]
                                .rearrange("(u b t) -> b u t", u=U1, b=B))

                if debug:
                    nc.sync.dma_start(pb_dbg.ap(), pbt[:, 0:1 + ROWS])
                    nc.sync.dma_start(pl_dbg.ap(), plt[:, 0:1 + ROWS])

                # ============ u-sweep DP ============
                e0 = fp.tile([8, T], F32, tag="e0", name="e0")
                nc.vector.memset(e0[:, :], 0.0)
                nc.vector.memset(e0[:, 0:1], 1.0)
                lr = fp.tile([8, U1], F32, tag="lr", name="lr")
                A = [fp.tile([8, T], F32, tag=f"A{i}", name=f"A{i}")
                     for i in range(2)]
                Dt = fp.tile([8, T], F32, tag="Dt", name="Dt")
                nc.vector.tensor_tensor_scan(
                    out=A[0][:, :], data0=pbt[:, 0:T], data1=e0[:, :],
                    initial=0.0, op0=OP.mult, op1=OP.add)
                cur = 0
                for u in range(1, U1):
                    nc.vector.tensor_reduce(
                        out=lr[:, u - 1:u], in_=A[cur][:, :],
                        axis=mybir.AxisListType.X, op=OP.max)
                    rc1 = fp.tile([8, 1], F32, tag="rc1", name="rc1")
                    nc.vector.reciprocal(rc1[:, :], lr[:, u - 1:u])
                    nc.vector.scalar_tensor_tensor(
                        out=Dt[:, :], in0=A[cur][:, :], scalar=rc1[:, 0:1],
                        in1=plt[:, 1 + (u - 1) * T:1 + u * T],
                        op0=OP.mult, op1=OP.mult)
                    nc.vector.tensor_tensor_scan(
                        out=A[1 - cur][:, :], data0=pbt[:, u * T:(u + 1) * T],
                        data1=Dt[:, :], initial=0.0, op0=OP.mult, op1=OP.add)
                    cur = 1 - cur
                nc.vector.tensor_tensor(
                    out=lr[:, U:U1], in0=A[cur][:, T - 1:T],
                    in1=pbt[:, 1 + U * T + T - 1:1 + U * T + T], op=OP.mult)
                lg = fp.tile([8, U1], F32, tag="lg", name="lg")
                nc.scalar.activation(lg[:, :], lr[:, :], AF.Ln)
                ssum = fp.tile([8, 1], F32, tag="ssum", name="ssum")
                nc.vector.tensor_reduce(out=ssum[:, :], in_=lg[:, :],
                                        axis=mybir.AxisListType.X, op=OP.add)
                tt = fp.tile([32, 32], F32, tag="tt", name="tt")
                nc.vector.memset(tt[:, :], 0.0)
                nc.vector.tensor_copy(tt[0:8, 0:1], ssum[:, :])
                tu = fp.tile([32, 32], F32, tag="tu", name="tu")
                nc.vector.transpose(tu[:, :], tt[:, :])
                tot = fp.tile([1, 1], F32, tag="tot", name="tot")
                nc.vector.tensor_reduce(out=tot[:, :], in_=tu[0:1, :],
                                        axis=mybir.AxisListType.X, op=OP.add)
                nc.vector.tensor_scalar(out=tot[:, :], in0=tot[:, :],
                                        scalar1=-0.125, scalar2=250.0 * C_SHIFT,
                                        op0=OP.mult, op1=OP.add)
                nc.sync.dma_start(loss_d.ap(), tot[:, :])

    nc.compile()
    return nc


def _host_prep(inputs):
    hs_pad = np.asarray(inputs["hs_pad"], np.float32)
    ys_pad = np.asarray(inputs["ys_pad"])
    embed_w = np.asarray(inputs["embed_w"], np.float32)
    loutb = np.asarray(inputs["lin_out_b"], np.float32)

    Emb = embed_w.copy()
    Emb[0] = 0.0
    ys_in = np.concatenate([np.zeros((B, 1), ys_pad.dtype), ys_pad], axis=1)
    eys = Emb[ys_in]                                   # (B, U1, D)
    # eysT[p, k*408 + u*8 + b] = eys[b, u, 128k+p]
    eysT = np.ascontiguousarray(
        eys.transpose(2, 1, 0).reshape(4, 128, U1, B)
        .transpose(1, 0, 2, 3).reshape(128, 4 * U1 * B)).astype(F8)

    def flip_w(w):
        W = np.asarray(w, np.float32)[PERM] * WS       # (2048, 512)
        A = W.reshape(16, 128, 4, 128).transpose(3, 2, 0, 1)
        return np.ascontiguousarray(A.reshape(128, 8192)).astype(F8)

    def flip_sq(w, scale, dt):
        A = (np.asarray(w, np.float32) * scale).reshape(4, 128, 4, 128)
        A = A.transpose(3, 2, 0, 1)
        return np.ascontiguousarray(A.reshape(128, 2048)).astype(dt)

    def bias_t(bi, bh):
        v = (np.asarray(bi, np.float32) + np.asarray(bh, np.float32))[PERM]
        return np.ascontiguousarray(v.reshape(16, 128).T.astype(np.float32))

    lout = np.asarray(inputs["lin_out_w"], np.float32)
    loutT = np.ascontiguousarray(
        (lout * WS).T.reshape(4, 128, 512).transpose(1, 0, 2)
        .reshape(128, 2048)).astype(F8)

    r = np.arange(RPAD)
    uu, bb = r // NTB, (r % NTB) // TSH
    valid = uu < U
    lab = np.where(valid, ys_pad[bb, np.minimum(uu, U - 1)], -1)
    ysidx = np.ascontiguousarray(
        lab.astype(np.float32).reshape(NMT, 128).T)
    eblab = np.where(valid, EC * np.exp(loutb[np.maximum(lab, 0)]), 1.0)
    eblab = np.ascontiguousarray(
        eblab.astype(np.float32).reshape(NMT, 128).T)

    common = {
        "eysT": eysT,
        "wih0": flip_w(inputs["w_ih0"]),
        "whh0": flip_w(inputs["w_hh0"]),
        "wih1": flip_w(inputs["w_ih1"]),
        "whh1": flip_w(inputs["w_hh1"]),
        "bias0": bias_t(inputs["b_ih0"], inputs["b_hh0"]),
        "bias1": bias_t(inputs["b_ih1"], inputs["b_hh1"]),
        "lencT": flip_sq(inputs["lin_enc_w"], 1.0, BF),
        "ldecT": flip_sq(inputs["lin_dec_w"], WS, F8),
        "loutT": loutT,
        "lencb": np.ascontiguousarray(
            np.asarray(inputs["lin_enc_b"], np.float32).reshape(4, 128).T),
        "ysidx": ysidx,
        "eblab": eblab,
        "ebc": np.full((128, 1), EC * np.exp(loutb[0]), np.float32),
        "ebrep": np.ascontiguousarray(
            np.tile(np.exp(loutb)[None, :], (128, 1)).astype(np.float32)),
    }

    in_maps = []
    for c in range(NCORES):
        hs = hs_pad[:, TSH * c:TSH * (c + 1), :]       # (B, 25, D)
        # hsT[p, k*200 + b*25 + t] = hs[b, t, 128k+p]
        hsT = np.ascontiguousarray(
            hs.transpose(2, 0, 1).reshape(4, 128, B, TSH)
            .transpose(1, 0, 2, 3).reshape(128, 4 * NTB)).astype(BF)
        m = dict(common)
        m["hsT"] = hsT
        in_maps.append(m)
    return in_maps


def kernel(**inputs):
    if "nc" not in _BUILD_CACHE:
        _BUILD_CACHE["nc"] = build(debug=False)
    nc = _BUILD_CACHE["nc"]
    in_maps = _host_prep(inputs)
    res = bass_utils.run_bass_kernel_spmd(nc, in_maps,
                                          core_ids=list(range(NCORES)))
    return np.float32(res.results[0]["loss"][0, 0])


if __name__ == "__main__":
    dat = np.load("/root/problem/inputs.npz")
    out = kernel(**{k: dat[k] for k in dat.files})
    exp = float(np.load("/root/problem/expected.npy"))
    rel = abs(float(out) - exp) / abs(exp)
    print("loss =", out, "expected =", exp, "rel err =", rel)
